# revision 22
# baseline (speedup 1.0000x reference)
"""Fused causal MHA block (QKV proj + 16-head attention + out proj) on 8 trn2 cores.

Sharding: core = (batch b in 0..3, head-group hg in 0..1); each core handles one
batch and 8 heads (512 of 1024 attention features). Host pre-tiles inputs to
contiguous f16 layouts so every DMA is descriptor-efficient.

v4 design: single software-pipelined timeline over seq chunks sc:
  slot sc: [pass1 row-maxima for all 4 head-pairs (64x128-tiled PE, packed),
            aug build, pass2 chunk sc for all 8 heads, per-slot 1/l
            normalization] with the NEXT chunk's QKV-projection matmuls (and
            the previous chunk's output projection) emitted as filler units
            between attention events, so the PE FIFO never stalls on the
            ScalarE exp or DVE reduce chains and HAM stays at K=8/8.
Softmax: pass1 computes only the exact row max m (no exp / Ln / accum);
pass2 folds m via a 65th aug row; P = exp(8(s-m)-40) in bf16; P@V appends a
ones-column to V so the true sum l accumulates in the same matmul;
out = (P@V) * (1/l) with reciprocal_approx_fast on DVE and a K=8
indicator-matrix matmul broadcasting 1/l across partitions.
"""
import sys
sys.path.insert(0, "/opt/trn_rl_repo")
import numpy as np

B, N, D = 4, 2048, 1024
H, DH = 16, 64
NCORES = 8
NEG = -1.0e9

_cache = {}


def _build(n=N):
    import concourse.bass as bass
    import concourse.tile as tile
    from concourse import bacc, mybir, masks
    from contextlib import ExitStack
    from collections import deque

    f32, f32r = mybir.dt.float32, mybir.dt.float32r
    f16, bf16 = mybir.dt.float16, mybir.dt.bfloat16
    AX, ALU, ACT = mybir.AxisListType, mybir.AluOpType, mybir.ActivationFunctionType

    nt = n // 128
    nsc = n // 512
    nk = 24
    nhp = 4

    nc = bacc.Bacc("TRN2", target_bir_lowering=False, debug=False,
                   num_devices=NCORES)
    xTt = nc.dram_tensor("xTt", [1024, 6144], f16, kind="ExternalInput").ap()
    wqkTt = nc.dram_tensor("wqkTt", [1024, 3072], f16, kind="ExternalInput").ap()
    wvT = nc.dram_tensor("wvT", [3072, 512], f16, kind="ExternalInput").ap()
    bqk = nc.dram_tensor("bqk", [128, 8], f32, kind="ExternalInput").ap()
    bv = nc.dram_tensor("bv", [128, 512], f32, kind="ExternalInput").ap()
    woT = nc.dram_tensor("woT", [512, 1024], bf16, kind="ExternalInput").ap()
    bo = nc.dram_tensor("bo", [128, 1024], f32, kind="ExternalInput").ap()
    efm = nc.dram_tensor("efm", [8, 512], bf16, kind="ExternalInput").ap()
    mask1 = nc.dram_tensor("mask1", [128, 128], f32, kind="ExternalInput").ap()
    maskT = nc.dram_tensor("maskT", [128, 128], f32, kind="ExternalInput").ap()
    out = nc.dram_tensor("out", [n, 1024], f32, kind="ExternalOutput").ap()

    with tile.TileContext(nc) as tc, ExitStack() as ctx:
        const = ctx.enter_context(tc.tile_pool(name="const", bufs=1))
        resid = ctx.enter_context(tc.tile_pool(name="resid", bufs=1))
        xtp = ctx.enter_context(tc.tile_pool(name="xtp", bufs=2))
        wftp = ctx.enter_context(tc.tile_pool(name="wft", bufs=2))
        wvp = ctx.enter_context(tc.tile_pool(name="wvp", bufs=2))
        ptp = ctx.enter_context(tc.tile_pool(name="ptp", bufs=2))
        statp = ctx.enter_context(tc.tile_pool(name="statp", bufs=2))
        outp = ctx.enter_context(tc.tile_pool(name="outp", bufs=2))
        qkp = ctx.enter_context(tc.tile_pool(name="qkp", bufs=1, space="PSUM"))
        vacp = ctx.enter_context(tc.tile_pool(name="vacp", bufs=1, space="PSUM"))
        widep = ctx.enter_context(tc.tile_pool(name="widep", bufs=2, space="PSUM"))
        pvp = ctx.enter_context(tc.tile_pool(name="pvp", bufs=2, space="PSUM"))

        # ---- constants ----
        ident = const.tile([128, 128], f32, tag="ident")
        masks.make_identity(nc, ident[:])
        m1 = const.tile([128, 128], f32, tag="m1")
        nc.sync.dma_start(m1[:], mask1)
        mTt = const.tile([128, 128], f32, tag="mT")
        nc.sync.dma_start(mTt[:], maskT)
        bqk_t = const.tile([128, 8], f32, tag="bqk")
        nc.sync.dma_start(bqk_t[:], bqk)
        bv_t = const.tile([128, 512], f32, tag="bv")
        nc.sync.dma_start(bv_t[:], bv)
        bo_t = const.tile([128, 1024], f32, tag="bo")
        nc.sync.dma_start(bo_t[:], bo)
        woT_t = const.tile([128, 4, 1024], bf16, tag="woT")
        for ft in range(4):
            nc.sync.dma_start(woT_t[:, ft, :], woT[128 * ft:128 * ft + 128, :])
        negones = const.tile([1, 512], f16, tag="negones")
        nc.vector.memset(negones[:], -1.0)
        nb40 = const.tile([128, 1], f32, tag="nb40")
        nc.vector.memset(nb40[:], -40.0)
        # ef[h, f, c] = 1 where head h owns partition c of feature-block f
        ef = const.tile([8, 4, 128], bf16, tag="ef")
        nc.sync.dma_start(ef[:], efm.rearrange("h (f c) -> h f c", c=128))

        # ---- residents ----
        qkT = resid.tile([128, 8, n], f16, tag="qkT")
        vv = resid.tile([128, nt, 8, 66], bf16, tag="vv")
        aoT = resid.tile([128, 4, n], bf16, tag="aoT")
        raccr = resid.tile([128, n], bf16, tag="raccr")
        lbuf = resid.tile([128, 512], f32, tag="lbuf")
        qaugs, kaugs = [], []
        for h in range(8):
            qa = resid.tile([65, n], f16, tag=f"qa{h}", name=f"qa{h}")
            ka = resid.tile([65, n], f16, tag=f"ka{h}", name=f"ka{h}")
            qaugs.append(qa)
            kaugs.append(ka)

        nc.vector.memset(vv[:, :, :, 64:65], 1.0)

        # ---------- filler machinery ----------
        filler = deque()

        def fill(k=1):
            for _ in range(k):
                if not filler:
                    return
                filler.popleft()()

        def flush():
            while filler:
                filler.popleft()()

        # ---------- phase1 (projection) for one sc: appended as filler ----
        def emit_phase1_units(sc):
            xts = []

            def u_xt():
                for half in range(2):
                    xt = xtp.tile([128, 12, 512], f16, tag="xt", name="xt")
                    r0 = (4 * half + sc) * 128
                    nc.sync.dma_start(
                        xt[:],
                        xTt[r0:r0 + 128, :].rearrange("p (c m) -> p c m", m=512))
                    xts.append(xt)
            u_xt()

            wfs = []

            def load_wf(ft):
                wf = wftp.tile([128, 24, 128], f16, tag="wf", name="wf")
                nc.sync.dma_start(
                    wf[:], wqkTt[128 * ft:128 * ft + 128, :].rearrange(
                        "p (c m) -> p c m", m=128))
                wfs.append(wf)
            load_wf(0)

            ctx_ps = []

            def u_qk(ft, k0, k1):
                def g():
                    if k0 == 0:
                        if ft < 7:
                            load_wf(ft + 1)
                        ps = qkp.tile([128, 512], f32, tag="qk", name="qk")
                        ctx_ps.append(ps)
                    ps = ctx_ps[-1]
                    for k in range(k0, k1):
                        nc.tensor.matmul(ps[:], wfs[ft][:, k, :],
                                         xts[k // 12][:, k % 12, :],
                                         start=(k == 0), stop=(k == nk - 1))
                    if k1 == nk:
                        nc.vector.tensor_scalar_add(
                            qkT[:, ft, 512 * sc:512 * sc + 512], ps[:],
                            bqk_t[:, ft:ft + 1])
                return g
            for ft in range(8):
                for k0 in range(0, nk, 6):
                    filler.append(u_qk(ft, k0, k0 + 6))

            vps = []

            def u_v(ss, k):
                if k == 0:
                    vps.clear()
                    vps.append(vacp.tile([128, 512], f32, tag="vac",
                                         name="vac"))
                wv_t = wvp.tile([128, 512], f16, tag="wv", name="wv")
                nc.sync.dma_start(wv_t[:], wvT[128 * k:128 * (k + 1), :])
                nc.tensor.matmul(
                    vps[0][:],
                    xts[k // 12][:, k % 12, 128 * ss:128 * ss + 128],
                    wv_t[:], start=(k == 0), stop=(k == nk - 1))
                if k == nk - 1:
                    t = 4 * sc + ss
                    for h in range(8):
                        nc.vector.tensor_add(
                            vv[:, t, h, 0:64],
                            vps[0][:, 64 * h:64 * h + 64],
                            bv_t[:, 64 * h:64 * h + 64])

            for ss in range(4):
                for k0 in range(0, nk, 3):
                    def g3(ss=ss, k0=k0):
                        for kk in range(k0, min(k0 + 3, nk)):
                            u_v(ss, kk)
                    filler.append(g3)

        # ---------- phase3 (out-proj) for one sc: appended as filler ----
        def emit_phase3_units(sc):
            for s in range(4 * sc, 4 * sc + 4):
                def g(s=s):
                    ps2 = widep.tile([128, 1024], f32, tag="wide", name="p3")
                    for oc in range(2):
                        for ft in range(4):
                            nc.tensor.matmul(
                                ps2[:, 512 * oc:512 * oc + 512],
                                aoT[:, ft, 128 * s:128 * s + 128],
                                woT_t[:, ft, 512 * oc:512 * oc + 512],
                                start=(ft == 0), stop=(ft == 3))
                    ot2 = outp.tile([128, 1024], f32, tag="ot", name="ot")
                    nc.vector.tensor_add(ot2[:], ps2[:], bo_t[:])
                    nc.sync.dma_start(out[128 * s:128 * s + 128, :], ot2[:])
                filler.append(g)

        # ---------- pass1: row maxima for slot sc, all head-pairs ----------
        def pass1_slot(sc, achS):
            for hp in range(nhp):
                ftq, ftk = hp, 4 + hp
                for it in range(4):
                    i = 4 * sc + it
                    nch = i // 4 + 1
                    rmx2 = statp.tile([128, 2, 4], f32, tag="rmx", bufs=4)
                    for jj in range(nch):
                        W = 512 if jj < i // 4 else 128 * (i % 4) + 128
                        wt = widep.tile([128, 1024], f32, tag="wide", name="p1")
                        wv2 = wt.rearrange("p (g w) -> p g w", g=2)
                        for gi, pb in enumerate((0, 64)):
                            nc.tensor.matmul(
                                wv2[:, gi, 0:W],
                                qkT[pb:pb + 64, ftq, 128 * i:128 * i + 128],
                                qkT[pb:pb + 64, ftk, 512 * jj:512 * jj + W],
                                start=True, stop=True)
                        for gi in range(2):
                            if jj == nch - 1:
                                nc.vector.tensor_add(
                                    wv2[:, gi, W - 128:W],
                                    wv2[:, gi, W - 128:W], m1[:])
                        nc.vector.tensor_reduce(rmx2[:, :, jj:jj + 1],
                                                wv2[:, :, 0:W], AX.X, ALU.max)
                        fill(1)
                    for gi in range(2):
                        nc.vector.tensor_reduce(
                            achS[:, 8 * hp + 4 * gi + it:
                                 8 * hp + 4 * gi + it + 1],
                            rmx2[:, gi, 0:nch], AX.X, ALU.max)
                fill(1)

        # ---------- aug build for slot sc ----------
        def aug_slot(sc, achS):
            tpp = qkp.tile([32, 128], f32, tag="qk", name="tpp")
            nc.tensor.transpose(tpp[:], achS[:, 0:32], ident[:])
            trow = statp.tile([32, 128], f16, tag="trow")
            nc.vector.tensor_copy(trow[:], tpp[:])
            cs = slice(512 * sc, 512 * sc + 512)
            for h in range(8):
                hp, h2 = h // 2, h % 2
                pb = 64 * h2
                ftq, ftk = hp, 4 + hp
                nc.gpsimd.dma_start(qaugs[h][0:64, cs],
                                    qkT[pb:pb + 64, ftq, cs])
                nc.gpsimd.dma_start(kaugs[h][0:64, cs],
                                    qkT[pb:pb + 64, ftk, cs])
                nc.gpsimd.dma_start(
                    qaugs[h][64:65, cs].rearrange("o (t f) -> o t f", f=128),
                    trow[8 * hp + 4 * h2:8 * hp + 4 * h2 + 4, :])
                nc.gpsimd.dma_start(kaugs[h][64:65, cs], negones[0:1, :])
                fill(1)

        # ---------- pass2 chunk c for head h ----------
        def plan_chunk(c):
            tiles, cur, fillw = [], [], 0
            for j in range(4 * c + 4):
                qs = max(512 * c, 128 * j)
                W = 512 * (c + 1) - qs
                if fillw + W > 1024:
                    tiles.append((cur, fillw))
                    cur, fillw = [], 0
                cur.append((j, qs, W, fillw))
                fillw += W
            tiles.append((cur, fillw))
            return tiles

        def pass2_chunk(h, c):
            pb, ftq = 64 * (h % 2), h // 2
            qaug, kaug = qaugs[h], kaugs[h]
            tiles = plan_chunk(c)
            nj = 4 * c + 4
            pv = pvp.tile([128, 512], f32, tag="pv")
            done = []
            for ti in range(len(tiles) + 1):
                if ti < len(tiles):
                    blocks, fillw = tiles[ti]
                    st2 = widep.tile([128, 1024], f32, tag="wide", name="st2")
                    for (j, qs, W, off) in blocks:
                        nc.tensor.matmul(st2[:, off:off + W],
                                         kaug[0:65, 128 * j:128 * j + 128],
                                         qaug[0:65, qs:qs + W],
                                         start=True, stop=True)
                    for (j, qs, W, off) in blocks:
                        if j >= 4 * c:
                            nc.vector.tensor_add(st2[:, off:off + 128],
                                                 st2[:, off:off + 128], mTt[:])
                    pt2 = ptp.tile([128, 1024], bf16, tag="pt")
                    nc.scalar.activation(pt2[:, 0:fillw], st2[:, 0:fillw],
                                         ACT.Exp, bias=nb40[:, 0:1], scale=8.0)
                    done.append((blocks, pt2))
                    fill(1)
                if ti >= 1:
                    blocks, pt2 = done[ti - 1]
                    for (j, qs, W, off) in blocks:
                        o0 = qs - 512 * c
                        nc.tensor.matmul(
                            pv[0:65, o0:o0 + W], vv[:, j, h, 0:65],
                            pt2[:, off:off + W],
                            start=(j == 0), stop=(j == nj - 1))
                    fill(1)
            nc.scalar.activation(aoT[pb:pb + 64, ftq, 512 * c:512 * c + 512],
                                 pv[0:64, 0:512], ACT.Copy)
            lst = statp.tile([128, 512], f32, tag="lst", bufs=1)
            nc.vector.tensor_copy(lst[64:65, :], pv[64:65, 0:512])
            nc.gpsimd.dma_start(lbuf[h:h + 1, 0:512], lst[64:65, :])

        # ---------- per-slot normalization (after all 8 heads) ----------
        def norm_slot(sc):
            cs = slice(512 * sc, 512 * sc + 512)
            scr = statp.tile([128, 512], f32, tag="scr", bufs=1)
            nc.vector.reciprocal_approx_fast(scr[0:8, :], lbuf[0:8, 0:512])
            nc.vector.tensor_copy(raccr[0:8, cs], scr[0:8, :])
            for f in range(4):
                bc = pvp.tile([128, 512], f32, tag="pv", name="bc")
                nc.tensor.matmul(bc[:], ef[0:8, f, :], raccr[0:8, cs],
                                 start=True, stop=True)
                nc.vector.tensor_mul(aoT[:, f, cs], aoT[:, f, cs], bc[:])
                fill(1)

        # ---------- timeline ----------
        emit_phase1_units(0)
        flush()
        emit_phase1_units(1)
        for sc in range(nsc):
            achS = statp.tile([128, 32], f32, tag="ach", bufs=2)
            pass1_slot(sc, achS)
            aug_slot(sc, achS)
            for h in range(8):
                pass2_chunk(h, sc)
                fill(1)
            norm_slot(sc)
            flush()  # phase1(sc+1) must complete before slot sc+1 reads qkT
            if sc + 2 < nsc:
                emit_phase1_units(sc + 2)
            emit_phase3_units(sc)
        flush()

    nc.compile()
    return nc


def _in_maps(q, k, v, w_qkv, b_qkv, w_out, b_out):
    import ml_dtypes
    x = np.concatenate([q, k, v], axis=-1)
    tri = np.triu(np.full((128, 128), NEG, np.float32), 1)
    maps = []
    for core in range(NCORES):
        b, hg = core // 2, core % 2
        fs = slice(512 * hg, 512 * hg + 512)
        wq = w_qkv[0 * D:1 * D][fs]
        wk = w_qkv[1 * D:2 * D][fs]
        wv = w_qkv[2 * D:3 * D][fs]
        bq = b_qkv[0 * D:1 * D][fs]
        bk = b_qkv[1 * D:2 * D][fs]
        bvb = b_qkv[2 * D:3 * D][fs]
        xT = np.ascontiguousarray(x[b].T)
        xtt = xT.reshape(2, 12, 128, 4, 512).transpose(0, 3, 2, 1, 4)
        xtt = np.ascontiguousarray(xtt.reshape(1024, 6144)).astype(np.float16)
        wqk = np.concatenate([wq, wk], 0).T
        wqt = wqk.reshape(24, 128, 8, 128).transpose(2, 1, 0, 3)
        wqt = np.ascontiguousarray(wqt.reshape(1024, 3072)).astype(np.float16)
        efm = np.zeros((8, 4, 128), np.float32)
        for f in range(4):
            efm[2 * f, f, 0:64] = 1.0
            efm[2 * f + 1, f, 64:128] = 1.0
        maps.append({
            "efm": efm.reshape(8, 512).astype(ml_dtypes.bfloat16),
            "xTt": xtt,
            "wqkTt": wqt,
            "wvT": np.ascontiguousarray(wv.T).astype(np.float16),
            "bqk": np.ascontiguousarray(
                np.concatenate([bq, bk]).reshape(8, 128).T),
            "bv": np.tile(bvb[None, :], (128, 1)),
            "woT": np.ascontiguousarray(w_out[:, fs].T).astype(
                ml_dtypes.bfloat16),
            "bo": np.tile(b_out[None, :], (128, 1)) if hg == 0
                  else np.zeros((128, D), np.float32),
            "mask1": tri,
            "maskT": np.ascontiguousarray(tri.T),
        })
    return maps


def kernel(q, k, v, w_qkv, b_qkv, w_out, b_out, _trace=False):
    from concourse import bass_utils
    if "nc" not in _cache:
        _cache["nc"] = _build()
    nc = _cache["nc"]
    maps = _in_maps(np.asarray(q, np.float32), np.asarray(k, np.float32),
                    np.asarray(v, np.float32), np.asarray(w_qkv, np.float32),
                    np.asarray(b_qkv, np.float32), np.asarray(w_out, np.float32),
                    np.asarray(b_out, np.float32))
    res = bass_utils.run_bass_kernel_spmd(nc, maps, core_ids=list(range(NCORES)),
                                          trace=_trace)
    outs = [np.asarray(res.results[c]["out"], np.float32) for c in range(NCORES)]
    full = np.stack([outs[2 * b] + outs[2 * b + 1] for b in range(B)], 0)
    if _trace:
        return full, res
    return full


# revision 23
# speedup vs baseline: 1.3240x; 1.3240x over previous
"""Fused causal MHA block (QKV proj + 16-head attention + out proj) on 8 trn2 cores.

Sharding: core = (batch b in 0..3, head-group hg in 0..1); each core handles one
batch and 8 heads (512 of 1024 attention features). Host pre-tiles inputs to
contiguous f16 layouts so every DMA is descriptor-efficient.

v4 design: single software-pipelined timeline over seq chunks sc:
  slot sc: [pass1 row-maxima for all 4 head-pairs (64x128-tiled PE, packed),
            aug build, pass2 chunk sc for all 8 heads, per-slot 1/l
            normalization] with the NEXT chunk's QKV-projection matmuls (and
            the previous chunk's output projection) emitted as filler units
            between attention events, so the PE FIFO never stalls on the
            ScalarE exp or DVE reduce chains and HAM stays at K=8/8.
Softmax: pass1 computes only the exact row max m (no exp / Ln / accum);
pass2 folds m via a 65th aug row; P = exp(8(s-m)-40) in bf16; P@V appends a
ones-column to V so the true sum l accumulates in the same matmul;
out = (P@V) * (1/l) with reciprocal_approx_fast on DVE and a K=8
indicator-matrix matmul broadcasting 1/l across partitions.
"""
import sys
sys.path.insert(0, "/opt/trn_rl_repo")
import numpy as np

B, N, D = 4, 2048, 1024
H, DH = 16, 64
NCORES = 8
NEG = -1.0e9

_cache = {}


def _build(n=N):
    import concourse.bass as bass
    import concourse.tile as tile
    from concourse import bacc, mybir, masks
    from contextlib import ExitStack
    from collections import deque

    f32, f32r = mybir.dt.float32, mybir.dt.float32r
    f16, bf16 = mybir.dt.float16, mybir.dt.bfloat16
    AX, ALU, ACT = mybir.AxisListType, mybir.AluOpType, mybir.ActivationFunctionType

    nt = n // 128
    nsc = n // 512
    nk = 24
    nhp = 4

    nc = bacc.Bacc("TRN2", target_bir_lowering=False, debug=False,
                   num_devices=NCORES)
    xTt = nc.dram_tensor("xTt", [1024, 6144], f16, kind="ExternalInput").ap()
    wqkTt = nc.dram_tensor("wqkTt", [1024, 3072], f16, kind="ExternalInput").ap()
    wvT = nc.dram_tensor("wvT", [3072, 512], f16, kind="ExternalInput").ap()
    bqk = nc.dram_tensor("bqk", [128, 8], f32, kind="ExternalInput").ap()
    bv = nc.dram_tensor("bv", [128, 512], f32, kind="ExternalInput").ap()
    woT = nc.dram_tensor("woT", [512, 1024], bf16, kind="ExternalInput").ap()
    bo = nc.dram_tensor("bo", [128, 1024], f32, kind="ExternalInput").ap()
    efm = nc.dram_tensor("efm", [8, 512], bf16, kind="ExternalInput").ap()
    mask1 = nc.dram_tensor("mask1", [128, 128], f32, kind="ExternalInput").ap()
    maskT = nc.dram_tensor("maskT", [128, 128], f32, kind="ExternalInput").ap()
    out = nc.dram_tensor("out", [n, 1024], f32, kind="ExternalOutput").ap()

    with tile.TileContext(nc) as tc, ExitStack() as ctx:
        const = ctx.enter_context(tc.tile_pool(name="const", bufs=1))
        resid = ctx.enter_context(tc.tile_pool(name="resid", bufs=1))
        xtp = ctx.enter_context(tc.tile_pool(name="xtp", bufs=2))
        wftp = ctx.enter_context(tc.tile_pool(name="wft", bufs=2))
        wvp = ctx.enter_context(tc.tile_pool(name="wvp", bufs=2))
        ptp = ctx.enter_context(tc.tile_pool(name="ptp", bufs=2))
        statp = ctx.enter_context(tc.tile_pool(name="statp", bufs=2))
        outp = ctx.enter_context(tc.tile_pool(name="outp", bufs=2))
        qkp = ctx.enter_context(tc.tile_pool(name="qkp", bufs=1, space="PSUM"))
        vacp = ctx.enter_context(tc.tile_pool(name="vacp", bufs=2, space="PSUM"))
        widep = ctx.enter_context(tc.tile_pool(name="widep", bufs=2, space="PSUM"))
        pvp = ctx.enter_context(tc.tile_pool(name="pvp", bufs=1, space="PSUM"))

        # ---- constants ----
        ident = const.tile([128, 128], f32, tag="ident")
        masks.make_identity(nc, ident[:])
        m1 = const.tile([128, 128], f32, tag="m1")
        nc.sync.dma_start(m1[:], mask1)
        mTt = const.tile([128, 128], f32, tag="mT")
        nc.sync.dma_start(mTt[:], maskT)
        bqk_t = const.tile([128, 8], f32, tag="bqk")
        nc.sync.dma_start(bqk_t[:], bqk)
        bv_t = const.tile([128, 512], f32, tag="bv")
        nc.sync.dma_start(bv_t[:], bv)
        bo_t = const.tile([128, 1024], f32, tag="bo")
        nc.sync.dma_start(bo_t[:], bo)
        woT_t = const.tile([128, 4, 1024], bf16, tag="woT")
        for ft in range(4):
            nc.sync.dma_start(woT_t[:, ft, :], woT[128 * ft:128 * ft + 128, :])
        negones = const.tile([1, 512], f16, tag="negones")
        nc.vector.memset(negones[:], -1.0)
        nb40 = const.tile([128, 1], f32, tag="nb40")
        nc.vector.memset(nb40[:], -40.0)
        # ef[h, f, c] = 1 where head h owns partition c of feature-block f
        ef = const.tile([8, 4, 128], bf16, tag="ef")
        nc.sync.dma_start(ef[:], efm.rearrange("h (f c) -> h f c", c=128))

        # ---- residents ----
        qkT = resid.tile([128, 8, n], f16, tag="qkT")
        vv = resid.tile([128, nt, 8, 66], bf16, tag="vv")
        aoT = resid.tile([128, 4, n], bf16, tag="aoT")
        raccr = resid.tile([128, n], bf16, tag="raccr")
        lbuf = resid.tile([128, 512], f32, tag="lbuf")
        qaugs, kaugs = [], []
        for h in range(8):
            qa = resid.tile([65, n], f16, tag=f"qa{h}", name=f"qa{h}")
            ka = resid.tile([65, n], f16, tag=f"ka{h}", name=f"ka{h}")
            qaugs.append(qa)
            kaugs.append(ka)

        nc.vector.memset(vv[:, :, :, 64:65], 1.0)

        # ---------- filler machinery ----------
        filler = deque()

        def fill(k=1):
            for _ in range(k):
                if not filler:
                    return
                filler.popleft()()

        def flush():
            while filler:
                filler.popleft()()

        # ---------- phase1 (projection) for one sc: appended as filler ----
        def emit_phase1_units(sc):
            xts = []

            def u_xt():
                for half in range(2):
                    xt = xtp.tile([128, 12, 512], f16, tag="xt", name="xt")
                    r0 = (4 * half + sc) * 128
                    nc.sync.dma_start(
                        xt[:],
                        xTt[r0:r0 + 128, :].rearrange("p (c m) -> p c m", m=512))
                    xts.append(xt)
            u_xt()

            wfs = []

            def load_wf(ft):
                wf = wftp.tile([128, 24, 128], f16, tag="wf", name="wf")
                nc.sync.dma_start(
                    wf[:], wqkTt[128 * ft:128 * ft + 128, :].rearrange(
                        "p (c m) -> p c m", m=128))
                wfs.append(wf)
            load_wf(0)

            ctx_ps = []

            def u_qk(ft, k0, k1):
                def g():
                    if k0 == 0:
                        if ft < 7:
                            load_wf(ft + 1)
                        ps = qkp.tile([128, 512], f32, tag="qk", name="qk")
                        ctx_ps.append(ps)
                    ps = ctx_ps[-1]
                    for k in range(k0, k1):
                        nc.tensor.matmul(ps[:], wfs[ft][:, k, :],
                                         xts[k // 12][:, k % 12, :],
                                         start=(k == 0), stop=(k == nk - 1))
                    if k1 == nk:
                        nc.vector.tensor_scalar_add(
                            qkT[:, ft, 512 * sc:512 * sc + 512], ps[:],
                            bqk_t[:, ft:ft + 1])
                return g
            for ft in range(8):
                for k0 in range(0, nk, 6):
                    filler.append(u_qk(ft, k0, k0 + 6))

            vps = []

            def u_v(rep, k):
                if k == 0:
                    vps.clear()
                    for _si in range(2):
                        vps.append(vacp.tile([128, 512], f32, tag="vac",
                                             name="vac"))
                wv_t = wvp.tile([128, 512], f16, tag="wv", name="wv")
                nc.sync.dma_start(wv_t[:], wvT[128 * k:128 * (k + 1), :])
                for si in range(2):
                    ss = 2 * rep + si
                    nc.tensor.matmul(
                        vps[si][:],
                        xts[k // 12][:, k % 12, 128 * ss:128 * ss + 128],
                        wv_t[:], start=(k == 0), stop=(k == nk - 1))
                if k == nk - 1:
                    for si in range(2):
                        ss = 2 * rep + si
                        t = 4 * sc + ss
                        for h in range(8):
                            nc.vector.tensor_add(
                                vv[:, t, h, 0:64],
                                vps[si][:, 64 * h:64 * h + 64],
                                bv_t[:, 64 * h:64 * h + 64])

            for rep in range(2):
                for k0 in range(0, nk, 2):
                    def g3(rep=rep, k0=k0):
                        for kk in range(k0, min(k0 + 2, nk)):
                            u_v(rep, kk)
                    filler.append(g3)

        # ---------- phase3 (out-proj) for one sc: appended as filler ----
        def emit_phase3_units(sc):
            for s in range(4 * sc, 4 * sc + 4):
                def g(s=s):
                    ps2 = widep.tile([128, 1024], f32, tag="wide", name="p3")
                    for oc in range(2):
                        for ft in range(4):
                            nc.tensor.matmul(
                                ps2[:, 512 * oc:512 * oc + 512],
                                aoT[:, ft, 128 * s:128 * s + 128],
                                woT_t[:, ft, 512 * oc:512 * oc + 512],
                                start=(ft == 0), stop=(ft == 3))
                    ot2 = outp.tile([128, 1024], f32, tag="ot", name="ot")
                    nc.vector.tensor_add(ot2[:], ps2[:], bo_t[:])
                    nc.sync.dma_start(out[128 * s:128 * s + 128, :], ot2[:])
                filler.append(g)

        # ---------- pass1: row maxima for slot sc, all head-pairs ----------
        def pass1_slot(sc, achS):
            for hp in range(nhp):
                ftq, ftk = hp, 4 + hp
                for it in range(4):
                    i = 4 * sc + it
                    nch = i // 4 + 1
                    rmx2 = statp.tile([128, 2, 4], f32, tag="rmx", bufs=4)
                    for jj in range(nch):
                        W = 512 if jj < i // 4 else 128 * (i % 4) + 128
                        wt = widep.tile([128, 1024], f32, tag="wide", name="p1")
                        wv2 = wt.rearrange("p (g w) -> p g w", g=2)
                        for gi, pb in enumerate((0, 64)):
                            nc.tensor.matmul(
                                wv2[:, gi, 0:W],
                                qkT[pb:pb + 64, ftq, 128 * i:128 * i + 128],
                                qkT[pb:pb + 64, ftk, 512 * jj:512 * jj + W],
                                start=True, stop=True)
                        for gi in range(2):
                            if jj == nch - 1:
                                nc.vector.tensor_add(
                                    wv2[:, gi, W - 128:W],
                                    wv2[:, gi, W - 128:W], m1[:])
                        nc.vector.tensor_reduce(rmx2[:, :, jj:jj + 1],
                                                wv2[:, :, 0:W], AX.X, ALU.max)
                        fill(1)
                    for gi in range(2):
                        nc.vector.tensor_reduce(
                            achS[:, 8 * hp + 4 * gi + it:
                                 8 * hp + 4 * gi + it + 1],
                            rmx2[:, gi, 0:nch], AX.X, ALU.max)
                fill(1)

        # ---------- aug build for slot sc ----------
        def aug_slot(sc, achS):
            tpp = qkp.tile([32, 128], f32, tag="qk", name="tpp")
            nc.tensor.transpose(tpp[:], achS[:, 0:32], ident[:])
            trow = statp.tile([32, 128], f16, tag="trow")
            nc.vector.tensor_copy(trow[:], tpp[:])
            cs = slice(512 * sc, 512 * sc + 512)
            for h in range(8):
                hp, h2 = h // 2, h % 2
                pb = 64 * h2
                ftq, ftk = hp, 4 + hp
                nc.gpsimd.dma_start(qaugs[h][0:64, cs],
                                    qkT[pb:pb + 64, ftq, cs])
                nc.gpsimd.dma_start(kaugs[h][0:64, cs],
                                    qkT[pb:pb + 64, ftk, cs])
                nc.gpsimd.dma_start(
                    qaugs[h][64:65, cs].rearrange("o (t f) -> o t f", f=128),
                    trow[8 * hp + 4 * h2:8 * hp + 4 * h2 + 4, :])
                nc.gpsimd.dma_start(kaugs[h][64:65, cs], negones[0:1, :])
                fill(1)

        # ---------- pass2 chunk c for head h ----------
        def plan_chunk(c):
            tiles, cur, fillw = [], [], 0
            for j in range(4 * c + 4):
                qs = max(512 * c, 128 * j)
                W = 512 * (c + 1) - qs
                if fillw + W > 1024:
                    tiles.append((cur, fillw))
                    cur, fillw = [], 0
                cur.append((j, qs, W, fillw))
                fillw += W
            tiles.append((cur, fillw))
            return tiles

        def pass2_chunk(h, c):
            pb, ftq = 64 * (h % 2), h // 2
            qaug, kaug = qaugs[h], kaugs[h]
            tiles = plan_chunk(c)
            nj = 4 * c + 4
            pv = pvp.tile([128, 512], f32, tag="pv")
            done = []
            for ti in range(len(tiles) + 1):
                if ti < len(tiles):
                    blocks, fillw = tiles[ti]
                    st2 = widep.tile([128, 1024], f32, tag="wide", name="st2")
                    for (j, qs, W, off) in blocks:
                        nc.tensor.matmul(st2[:, off:off + W],
                                         kaug[0:65, 128 * j:128 * j + 128],
                                         qaug[0:65, qs:qs + W],
                                         start=True, stop=True)
                    for (j, qs, W, off) in blocks:
                        if j >= 4 * c:
                            nc.vector.tensor_add(st2[:, off:off + 128],
                                                 st2[:, off:off + 128], mTt[:])
                    pt2 = ptp.tile([128, 1024], bf16, tag="pt")
                    nc.scalar.activation(pt2[:, 0:fillw], st2[:, 0:fillw],
                                         ACT.Exp, bias=nb40[:, 0:1], scale=8.0)
                    done.append((blocks, pt2))
                    fill(1)
                if ti >= 1:
                    blocks, pt2 = done[ti - 1]
                    for (j, qs, W, off) in blocks:
                        o0 = qs - 512 * c
                        nc.tensor.matmul(
                            pv[0:65, o0:o0 + W], vv[:, j, h, 0:65],
                            pt2[:, off:off + W],
                            start=(j == 0), stop=(j == nj - 1))
                    fill(1)
            nc.scalar.activation(aoT[pb:pb + 64, ftq, 512 * c:512 * c + 512],
                                 pv[0:64, 0:512], ACT.Copy)
            lst = statp.tile([128, 512], f32, tag="lst", bufs=1)
            nc.vector.tensor_copy(lst[64:65, :], pv[64:65, 0:512])
            nc.gpsimd.dma_start(lbuf[h:h + 1, 0:512], lst[64:65, :])

        # ---------- per-slot normalization (after all 8 heads) ----------
        def norm_slot(sc):
            cs = slice(512 * sc, 512 * sc + 512)
            scr = statp.tile([128, 512], f32, tag="scr", bufs=1)
            nc.vector.reciprocal_approx_fast(scr[0:8, :], lbuf[0:8, 0:512])
            nc.vector.tensor_copy(raccr[0:8, cs], scr[0:8, :])
            for f in range(4):
                bc = pvp.tile([128, 512], f32, tag="pv", name="bc")
                nc.tensor.matmul(bc[:], ef[0:8, f, :], raccr[0:8, cs],
                                 start=True, stop=True)
                nc.vector.tensor_mul(aoT[:, f, cs], aoT[:, f, cs], bc[:])
                fill(1)

        # ---------- timeline ----------
        emit_phase1_units(0)
        flush()
        emit_phase1_units(1)
        for sc in range(nsc):
            achS = statp.tile([128, 32], f32, tag="ach", bufs=2)
            pass1_slot(sc, achS)
            aug_slot(sc, achS)
            for h in range(8):
                pass2_chunk(h, sc)
                fill(1)
            norm_slot(sc)
            flush()  # phase1(sc+1) must complete before slot sc+1 reads qkT
            if sc + 2 < nsc:
                emit_phase1_units(sc + 2)
            emit_phase3_units(sc)
        flush()

    nc.compile()
    return nc


def _in_maps(q, k, v, w_qkv, b_qkv, w_out, b_out):
    import ml_dtypes
    x = np.concatenate([q, k, v], axis=-1)
    tri = np.triu(np.full((128, 128), NEG, np.float32), 1)
    maps = []
    for core in range(NCORES):
        b, hg = core // 2, core % 2
        fs = slice(512 * hg, 512 * hg + 512)
        wq = w_qkv[0 * D:1 * D][fs]
        wk = w_qkv[1 * D:2 * D][fs]
        wv = w_qkv[2 * D:3 * D][fs]
        bq = b_qkv[0 * D:1 * D][fs]
        bk = b_qkv[1 * D:2 * D][fs]
        bvb = b_qkv[2 * D:3 * D][fs]
        xT = np.ascontiguousarray(x[b].T)
        xtt = xT.reshape(2, 12, 128, 4, 512).transpose(0, 3, 2, 1, 4)
        xtt = np.ascontiguousarray(xtt.reshape(1024, 6144)).astype(np.float16)
        wqk = np.concatenate([wq, wk], 0).T
        wqt = wqk.reshape(24, 128, 8, 128).transpose(2, 1, 0, 3)
        wqt = np.ascontiguousarray(wqt.reshape(1024, 3072)).astype(np.float16)
        efm = np.zeros((8, 4, 128), np.float32)
        for f in range(4):
            efm[2 * f, f, 0:64] = 1.0
            efm[2 * f + 1, f, 64:128] = 1.0
        maps.append({
            "efm": efm.reshape(8, 512).astype(ml_dtypes.bfloat16),
            "xTt": xtt,
            "wqkTt": wqt,
            "wvT": np.ascontiguousarray(wv.T).astype(np.float16),
            "bqk": np.ascontiguousarray(
                np.concatenate([bq, bk]).reshape(8, 128).T),
            "bv": np.tile(bvb[None, :], (128, 1)),
            "woT": np.ascontiguousarray(w_out[:, fs].T).astype(
                ml_dtypes.bfloat16),
            "bo": np.tile(b_out[None, :], (128, 1)) if hg == 0
                  else np.zeros((128, D), np.float32),
            "mask1": tri,
            "maskT": np.ascontiguousarray(tri.T),
        })
    return maps


def kernel(q, k, v, w_qkv, b_qkv, w_out, b_out, _trace=False):
    from concourse import bass_utils
    if "nc" not in _cache:
        _cache["nc"] = _build()
    nc = _cache["nc"]
    maps = _in_maps(np.asarray(q, np.float32), np.asarray(k, np.float32),
                    np.asarray(v, np.float32), np.asarray(w_qkv, np.float32),
                    np.asarray(b_qkv, np.float32), np.asarray(w_out, np.float32),
                    np.asarray(b_out, np.float32))
    res = bass_utils.run_bass_kernel_spmd(nc, maps, core_ids=list(range(NCORES)),
                                          trace=_trace)
    outs = [np.asarray(res.results[c]["out"], np.float32) for c in range(NCORES)]
    full = np.stack([outs[2 * b] + outs[2 * b + 1] for b in range(B)], 0)
    if _trace:
        return full, res
    return full
